# revision 1
# baseline (speedup 1.0000x reference)
"""CrossViewFusion Trainium2 kernel (v2: fp8 DoubleRow q/k/gate + shared-k).

Math (per batch row b):
  seq = [x_cc; x_mlo]                  # 2 views, D=512 each
  qkv = seq @ in_proj_w.T + b          # per view: q,k,v (512 each, 8 heads x 64)
  scores[h,qv,kv] = q_qv[h] . k_kv[h] / 8
  key mlo masked out when view_mask[:,1]==0 -> softmax over 2 keys
  ao = attn @ v ; proj = ao @ out_w.T + out_b
  h = seq + proj ; t = LN(h) (gamma/beta = ln_g/ln_b)
  g = sigmoid([t_cc*g+b ; t_mlo*g+b] @ gate_w.T + gate_b)
  fused = g*cc + (1-g)*mlo ; out = has_mlo ? fused : cc   (has_cc==1)

Implementation notes (per core, B/8 = 8192 rows, groups of 128 rows):
  - softmax over 2 keys folded to sigmoid on s = q.(k_cc - k_mlo)/8;
    dk/dv computed once from x_d = x_cc - x_mlo (saves one k matmul)
  - fp8e4 DoubleRow (2x PE throughput) for the error-tolerant matmuls:
    q_cc/q_mlo/dk (score path, passes through sigmoid) and the gate
    (passes through sigmoid then blends); weights pre-scaled by 64 on
    host, descale folded into the sigmoid activation scale
  - v/out matmuls stay bf16 (their error hits the output directly)
  - bf16 x uploaded from host (halves HBM + kills f32->bf16 casts)
  - residual+row-sum fused via scalar_tensor_tensor accum_out; sumsq via
    ACT Square accum; LN affine on ACT with per-row scale/bias APs
  - fp8 casts spread across ACT/POOL/DVE; final blend on POOL
"""

import sys

for _p in ("/opt/trn_rl_repo",):
    if _p not in sys.path:
        sys.path.append(_p)

import numpy as np
import ml_dtypes

B, D, H, HD = 65536, 512, 8, 64
NCORES = 8
BS = B // NCORES          # rows per core
P = 128                   # SBUF partitions
SG = 2                    # row-groups per super-group
EPS = 1e-5
WS = 64.0                 # fp8 weight pre-scale

BF16 = ml_dtypes.bfloat16
F8 = ml_dtypes.float8_e4m3

_cache = {}


def _build(flags, bs=BS, repeats=1):
    """Build + compile the per-core Bass kernel. flags =
    (zero_ipb, zero_ob, zero_gb2, unit_lng, zero_lnb, mask_binary).
    repeats>1 wraps the body in a hardware loop (benchmarking only)."""
    import concourse.mybir as mybir
    from concourse import bacc, tile
    from concourse.bass import ts
    from contextlib import ExitStack

    zero_ipb, zero_ob, zero_gb2, unit_lng, zero_lnb, mask_binary = flags
    blend_then_affine = zero_lnb or mask_binary

    nsg = bs // (P * SG)
    f32 = mybir.dt.float32
    bf16 = mybir.dt.bfloat16
    fp8 = mybir.dt.float8e4
    AF = mybir.ActivationFunctionType
    OP = mybir.AluOpType
    AX = mybir.AxisListType
    DR = mybir.MatmulPerfMode.DoubleRow

    nc = bacc.Bacc("TRN2", target_bir_lowering=False, debug=False,
                   enable_asserts=False)

    # ---- DRAM I/O ----
    x_cc_d = nc.dram_tensor("x_cc", [bs, D], bf16, kind="ExternalInput").ap()
    x_mlo_d = nc.dram_tensor("x_mlo", [bs, D], bf16, kind="ExternalInput").ap()
    vm_d = nc.dram_tensor("vm", [bs, 2], f32, kind="ExternalInput").ap()
    wqT8_d = nc.dram_tensor("wqT8", [D, D], fp8, kind="ExternalInput").ap()
    wkT8_d = nc.dram_tensor("wkT8", [D, D], fp8, kind="ExternalInput").ap()
    wvT_d = nc.dram_tensor("wvT", [D, D], bf16, kind="ExternalInput").ap()
    woT_d = nc.dram_tensor("woT", [D, D], bf16, kind="ExternalInput").ap()
    gwT8_d = nc.dram_tensor("gwT8", [2 * D, D], fp8, kind="ExternalInput").ap()
    opt_in = {}
    if not zero_ipb:
        # q bias pre-scaled by WS (added to scaled q psum); k bias cancels in dk
        opt_in["ipbq"] = nc.dram_tensor("ipbq", [D], f32, kind="ExternalInput").ap()
        opt_in["ipbv"] = nc.dram_tensor("ipbv", [D], f32, kind="ExternalInput").ap()
    if not zero_ob:
        opt_in["ob"] = nc.dram_tensor("ob", [D], f32, kind="ExternalInput").ap()
    if not zero_gb2:
        # pre-scaled by WS
        opt_in["gb2"] = nc.dram_tensor("gb2", [D], f32, kind="ExternalInput").ap()
    if not unit_lng:
        opt_in["lng"] = nc.dram_tensor("lng", [D], f32, kind="ExternalInput").ap()
    if not zero_lnb:
        opt_in["lnb"] = nc.dram_tensor("lnb", [D], f32, kind="ExternalInput").ap()
    out_d = nc.dram_tensor("out", [bs, D], bf16, kind="ExternalOutput").ap()

    with tile.TileContext(nc) as tc, ExitStack() as ctx:
        wpool = ctx.enter_context(tc.tile_pool(name="wpool", bufs=1))
        sb = ctx.enter_context(tc.tile_pool(name="sb", bufs=2))
        ps_qk = ctx.enter_context(tc.tile_pool(name="ps_qk", bufs=4, space="PSUM"))
        ps_v = ctx.enter_context(tc.tile_pool(name="ps_v", bufs=2, space="PSUM"))
        ps_pg = ctx.enter_context(tc.tile_pool(name="ps_pg", bufs=2, space="PSUM"))

        # ---- resident weights ----
        wqT8_sb = wpool.tile([P, 4, D], fp8)
        nc.sync.dma_start(wqT8_sb[:], wqT8_d.rearrange("(c p) f -> p c f", p=P))
        wkT8_sb = wpool.tile([P, 4, D], fp8)
        nc.sync.dma_start(wkT8_sb[:], wkT8_d.rearrange("(c p) f -> p c f", p=P))
        wvT_sb = wpool.tile([P, 4, D], bf16)
        nc.sync.dma_start(wvT_sb[:], wvT_d.rearrange("(c p) f -> p c f", p=P))
        woT_sb = wpool.tile([P, 4, D], bf16)
        nc.sync.dma_start(woT_sb[:], woT_d.rearrange("(c p) f -> p c f", p=P))
        gwT8_sb = wpool.tile([P, 8, D], fp8)
        nc.sync.dma_start(gwT8_sb[:], gwT8_d.rearrange("(c p) f -> p c f", p=P))

        def bcast_tile(name, dram_ap, n):
            t32 = wpool.tile([P, n], f32, name=name + "_f32")
            nc.sync.dma_start(t32[:], dram_ap[None, :].to_broadcast((P, n)))
            return t32

        eps_p1 = wpool.tile([P, 1], f32)
        nc.vector.memset(eps_p1[:], EPS)

        # PE warmup: scratch matmuls with no DMA dependency keep the HAM
        # clock-gate warm while the weight loads land; results discarded.
        wu_s = wpool.tile([P, D], bf16)
        nc.vector.memset(wu_s[:], 0)
        if repeats == 1:
            wu_ps = ps_pg.tile([P, D], f32, name="wu_ps", tag="pg")
            for _ in range(20):
                nc.tensor.matmul(wu_ps[:], wu_s[:, 0:P], wu_s[:],
                                 start=True, stop=True)

        ipbq_bc = None if zero_ipb else bcast_tile("ipbq_bc", opt_in["ipbq"], D)
        ipbv_bc = None if zero_ipb else bcast_tile("ipbv_bc", opt_in["ipbv"], D)
        ob_bc = None if zero_ob else bcast_tile("ob_bc", opt_in["ob"], D)
        gb2_bc = None if zero_gb2 else bcast_tile("gb2_bc", opt_in["gb2"], D)
        lng_bc = None if unit_lng else bcast_tile("lng_bc", opt_in["lng"], D)
        lnb_bc = None if zero_lnb else bcast_tile("lnb_bc", opt_in["lnb"], D)

        rep_cm = tc.For_i(0, repeats, 1) if repeats > 1 else None
        if rep_cm is not None:
            rep_cm.__enter__()

        for s in range(nsg):
            rows = ts(s, P * SG)
            # ---- batched loads (bf16 x direct from HBM) ----
            xccB = sb.tile([P, SG, D], bf16, bufs=3)
            nc.sync.dma_start(xccB[:], x_cc_d[rows, :].rearrange(
                "(n p) d -> p n d", p=P))
            xmloB = sb.tile([P, SG, D], bf16, bufs=3)
            nc.sync.dma_start(xmloB[:], x_mlo_d[rows, :].rearrange(
                "(n p) d -> p n d", p=P))
            vmB = sb.tile([P, SG, 2], f32)
            nc.sync.dma_start(vmB[:], vm_d[rows, :].rearrange(
                "(n p) c -> p n c", p=P))

            # ---- per-row mask scalars, [128, SG] ----
            a4 = vmB[:, :, 0]
            m4 = vmB[:, :, 1]
            bm4 = sb.tile([P, SG], f32)
            nc.vector.tensor_scalar(bm4[:], m4, 0.0, None, op0=OP.not_equal)
            onemb4 = sb.tile([P, SG], f32)
            nc.vector.tensor_scalar(onemb4[:], bm4[:], -1.0, 1.0, op0=OP.mult,
                                    op1=OP.add)
            am4 = sb.tile([P, SG], f32)
            nc.vector.tensor_mul(am4[:], a4, m4)
            c4 = sb.tile([P, SG], f32)
            nc.vector.tensor_scalar(c4[:], am4[:], 0.5, None, op0=OP.is_gt)
            u4 = sb.tile([P, SG], f32)
            nc.vector.tensor_scalar(u4[:], c4[:], -1.0, 1.0, op0=OP.mult,
                                    op1=OP.add)
            scc24 = sb.tile([P, SG], f32)
            nc.vector.tensor_mul(scc24[:], u4[:], a4)
            negc4 = sb.tile([P, SG], f32)
            nc.vector.tensor_scalar(negc4[:], c4[:], -1.0, None, op0=OP.mult)
            mu4 = sb.tile([P, SG], f32)
            nc.vector.tensor_mul(mu4[:], m4, u4[:])
            smlo24 = sb.tile([P, SG], f32)
            nc.vector.tensor_add(smlo24[:], mu4[:], c4[:])

            # ---- x_d = x_cc - x_mlo; transposes; fp8 casts ----
            xdB = sb.tile([P, SG, D], bf16)
            nc.vector.tensor_sub(xdB[:], xccB[:], xmloB[:])
            xT_cc = sb.tile([P, SG * 4, P], bf16, bufs=3)
            nc.sync.dma_start_transpose(
                xT_cc[:], xccB[:].rearrange("p n d -> p (n d)"))
            xT_mlo = sb.tile([P, SG * 4, P], bf16, bufs=3)
            nc.sync.dma_start_transpose(
                xT_mlo[:], xmloB[:].rearrange("p n d -> p (n d)"))
            xT_d = sb.tile([P, SG * 4, P], bf16, bufs=3)
            nc.sync.dma_start_transpose(
                xT_d[:], xdB[:].rearrange("p n d -> p (n d)"))
            xT_cc8 = sb.tile([P, SG * 4, P], fp8, bufs=3)
            nc.scalar.copy(xT_cc8[:], xT_cc[:])
            xT_mlo8 = sb.tile([P, SG * 4, P], fp8, bufs=3)
            nc.gpsimd.tensor_copy(xT_mlo8[:], xT_mlo[:])
            xT_d8 = sb.tile([P, SG * 4, P], fp8, bufs=3)
            nc.vector.tensor_copy(xT_d8[:], xT_d[:])

            # big tiles for this super-group
            prodsB = sb.tile([P, SG, 2, D], bf16)
            sAllB = sb.tile([P, SG, 2, H], f32)
            wsigB = sb.tile([P, SG, 2, H], bf16)
            weffB = sb.tile([P, SG, 2, H], bf16)
            aoB = [sb.tile([P, SG, D], bf16, name=f"ao_{nm}")
                   for nm in ("cc", "mlo")]
            hB = [sb.tile([P, SG, D], bf16, name=f"h_{nm}")
                  for nm in ("cc", "mlo")]
            hsum8 = sb.tile([P, 2 * SG], f32)
            sq8 = sb.tile([P, 2 * SG], f32)
            tB = [sb.tile([P, SG, D], bf16, name=f"t_{nm}")
                  for nm in ("cc", "mlo")]
            gsigB = sb.tile([P, SG, D], bf16)
            bccB = sb.tile([P, SG, D], bf16)
            bmlB = sb.tile([P, SG, D], bf16)
            ofinB = sb.tile([P, SG, D], bf16)

            # ---- in_proj + scores + ao per group ----
            for n in range(SG):
                pq_cc = ps_qk.tile([P, D], f32, name="pq_cc", tag="qk")
                for c in (0, 2):
                    nc.tensor.matmul(pq_cc[:], xT_cc8[:, 4 * n + c:4 * n + c + 2, :],
                                     wqT8_sb[:, c:c + 2, :],
                                     start=(c == 0), stop=(c == 2), perf_mode=DR)
                pq_ml = ps_qk.tile([P, D], f32, name="pq_ml", tag="qk")
                for c in (0, 2):
                    nc.tensor.matmul(pq_ml[:], xT_mlo8[:, 4 * n + c:4 * n + c + 2, :],
                                     wqT8_sb[:, c:c + 2, :],
                                     start=(c == 0), stop=(c == 2), perf_mode=DR)
                pdk = ps_qk.tile([P, D], f32, name="pdk", tag="qk")
                for c in (0, 2):
                    nc.tensor.matmul(pdk[:], xT_d8[:, 4 * n + c:4 * n + c + 2, :],
                                     wkT8_sb[:, c:c + 2, :],
                                     start=(c == 0), stop=(c == 2), perf_mode=DR)
                if not zero_ipb:
                    nc.vector.tensor_add(pq_cc[:], pq_cc[:], ipbq_bc[:])
                    nc.vector.tensor_add(pq_ml[:], pq_ml[:], ipbq_bc[:])
                pdv = ps_v.tile([P, D], f32, name="pdv", tag="v")
                for c in range(4):
                    nc.tensor.matmul(pdv[:], xT_d[:, 4 * n + c, :],
                                     wvT_sb[:, c, :],
                                     start=(c == 0), stop=(c == 3))
                pvm = ps_v.tile([P, D], f32, name="pvm", tag="v")
                for c in range(4):
                    nc.tensor.matmul(pvm[:], xT_mlo[:, 4 * n + c, :],
                                     wvT_sb[:, c, :],
                                     start=(c == 0), stop=(c == 3))
                if not zero_ipb:
                    nc.vector.tensor_add(pvm[:], pvm[:], ipbv_bc[:])

                # scores: s_v = sum_h q_v . dk ; w = sigmoid(s/(8*WS^2))
                # (drain dk first: DVE can't read two PSUM operands)
                dkS = sb.tile([P, D], bf16, name="dkS", tag="dkS", bufs=2)
                nc.scalar.copy(dkS[:], pdk[:])
                nc.vector.tensor_mul(prodsB[:, n, 0, :], pq_cc[:], dkS[:])
                nc.vector.tensor_mul(prodsB[:, n, 1, :], pq_ml[:], dkS[:])
                nc.vector.reduce_sum(
                    sAllB[:, n],
                    prodsB[:, n].rearrange("p a (h e) -> p (a h) e", e=HD),
                    axis=AX.X)
                nc.scalar.activation(
                    wsigB[:, n].rearrange("p a h -> p (a h)"),
                    sAllB[:, n].rearrange("p a h -> p (a h)"),
                    AF.Sigmoid, scale=1.0 / (np.sqrt(HD) * WS * WS))
                # w_eff = bm*w + (1-bm): masked rows -> 1 (all weight on cc)
                nc.vector.tensor_scalar(
                    weffB[:, n].rearrange("p a h -> p (a h)"),
                    wsigB[:, n].rearrange("p a h -> p (a h)"),
                    bm4[:, n:n + 1], onemb4[:, n:n + 1],
                    op0=OP.mult, op1=OP.add)

                # ao_v = v_mlo + w_v * dv
                for vi in range(2):
                    wdv = sb.tile([P, D], bf16, name="wdv", tag="wdv", bufs=2)
                    nc.vector.tensor_mul(
                        wdv[:].rearrange("p (h e) -> p h e", e=HD),
                        pdv[:].rearrange("p (h e) -> p h e", e=HD),
                        weffB[:, n, vi, :].unsqueeze(2).broadcast_to((P, H, HD)))
                    nc.vector.tensor_add(aoB[vi][:, n, :], wdv[:], pvm[:])

            # ---- out_proj + residual + LN ----
            aoTs = []
            for vi, nm in ((0, "cc"), (1, "mlo")):
                aoT = sb.tile([P, SG * 4, P], bf16, name=f"aoT_{nm}")
                nc.sync.dma_start_transpose(
                    aoT[:], aoB[vi][:].rearrange("p n d -> p (n d)"))
                aoTs.append(aoT)
            for n in range(SG):
                for vi in range(2):
                    po = ps_pg.tile([P, D], f32, name="po", tag="pg")
                    for c in range(4):
                        nc.tensor.matmul(po[:], aoTs[vi][:, 4 * n + c, :],
                                         woT_sb[:, c, :],
                                         start=(c == 0), stop=(c == 3))
                    if not zero_ob:
                        nc.vector.tensor_add(po[:], po[:], ob_bc[:])
                    xV = (xccB, xmloB)[vi]
                    i8 = vi * SG + n
                    # h = x + o, with row-sum accumulated in the same op
                    nc.vector.scalar_tensor_tensor(
                        hB[vi][:, n, :], po[:], 1.0, xV[:, n, :],
                        op0=OP.mult, op1=OP.add,
                        accum_out=hsum8[:, i8:i8 + 1])
                    h2s = sb.tile([P, D], bf16, name="h2s", tag="h2s", bufs=2)
                    nc.scalar.activation(h2s[:], hB[vi][:, n, :],
                                         AF.Square,
                                         accum_out=sq8[:, i8:i8 + 1])

            # LN stats (tiny [P, 2*SG] ops)
            mneg8 = sb.tile([P, 2 * SG], f32)
            nc.vector.tensor_scalar(mneg8[:], hsum8[:], -1.0 / D, None,
                                    op0=OP.mult)
            ex28 = sb.tile([P, 2 * SG], f32)
            nc.vector.tensor_scalar(ex28[:], sq8[:], 1.0 / D, None, op0=OP.mult)
            var8 = sb.tile([P, 2 * SG], f32)
            nc.vector.tensor_mul(var8[:], mneg8[:], mneg8[:])
            nc.vector.tensor_sub(var8[:], ex28[:], var8[:])
            std8 = sb.tile([P, 2 * SG], f32)
            nc.scalar.activation(std8[:], var8[:], AF.Sqrt, bias=eps_p1[:])
            rs8 = sb.tile([P, 2 * SG], f32)
            nc.vector.reciprocal(rs8[:], std8[:])
            nmrs8 = sb.tile([P, 2 * SG], f32)
            nc.vector.tensor_mul(nmrs8[:], mneg8[:], rs8[:])

            # t = h*rs + nm on ACT (per-row scale/bias APs)
            for vi in range(2):
                for n in range(SG):
                    i8 = vi * SG + n
                    nc.scalar.activation(tB[vi][:, n, :], hB[vi][:, n, :],
                                         AF.Identity,
                                         scale=rs8[:, i8:i8 + 1],
                                         bias=nmrs8[:, i8:i8 + 1])

            # ---- gate (fp8 DoubleRow, K=1024) ----
            tT8s = []
            for vi, nm in ((0, "cc"), (1, "mlo")):
                tT = sb.tile([P, SG * 4, P], bf16, name=f"tT_{nm}")
                nc.sync.dma_start_transpose(
                    tT[:], tB[vi][:].rearrange("p n d -> p (n d)"))
                tT8 = sb.tile([P, SG * 4, P], fp8, name=f"tT8_{nm}")
                nc.gpsimd.tensor_copy(tT8[:], tT[:])
                tT8s.append(tT8)
            for n in range(SG):
                pg = ps_pg.tile([P, D], f32, name="pgate", tag="pg")
                for c in (0, 2):
                    nc.tensor.matmul(pg[:], tT8s[0][:, 4 * n + c:4 * n + c + 2, :],
                                     gwT8_sb[:, c:c + 2, :],
                                     start=(c == 0), stop=False, perf_mode=DR)
                for c in (0, 2):
                    nc.tensor.matmul(pg[:], tT8s[1][:, 4 * n + c:4 * n + c + 2, :],
                                     gwT8_sb[:, 4 + c:4 + c + 2, :],
                                     start=False, stop=(c == 2), perf_mode=DR)
                if not zero_gb2:
                    nc.vector.tensor_add(pg[:], pg[:], gb2_bc[:])
                nc.scalar.activation(gsigB[:, n, :], pg[:], AF.Sigmoid,
                                     scale=1.0 / WS)
                nc.vector.tensor_scalar(bccB[:, n, :], gsigB[:, n, :],
                                        c4[:, n:n + 1], scc24[:, n:n + 1],
                                        op0=OP.mult, op1=OP.add)
                nc.vector.tensor_scalar(bmlB[:, n, :], gsigB[:, n, :],
                                        negc4[:, n:n + 1], smlo24[:, n:n + 1],
                                        op0=OP.mult, op1=OP.add)

            # ---- final blend (gpsimd) ----
            if blend_then_affine:
                o1 = sb.tile([P, SG, D], bf16)
                nc.gpsimd.tensor_mul(o1[:], bccB[:], tB[0][:])
                o2 = sb.tile([P, SG, D], bf16)
                nc.gpsimd.tensor_mul(o2[:], bmlB[:], tB[1][:])
                need_post = (not unit_lng) or (not zero_lnb)
                if not need_post:
                    nc.gpsimd.tensor_add(ofinB[:], o1[:], o2[:])
                else:
                    osum = sb.tile([P, SG, D], f32, name="osum")
                    nc.gpsimd.tensor_add(osum[:], o1[:], o2[:])
                    cur = osum
                    if not unit_lng:
                        for n in range(SG):
                            nc.vector.tensor_mul(cur[:, n, :], cur[:, n, :],
                                                 lng_bc[:])
                    if not zero_lnb:
                        for n in range(SG):
                            nc.vector.tensor_add(ofinB[:, n, :], cur[:, n, :],
                                                 lnb_bc[:])
                    else:
                        nc.vector.tensor_copy(ofinB[:], cur[:])
            else:
                fins = []
                for vi in range(2):
                    fv = sb.tile([P, SG, D], f32, name=f"fin{vi}")
                    for n in range(SG):
                        cur_in = tB[vi][:, n, :]
                        if not unit_lng:
                            nc.vector.tensor_mul(fv[:, n, :], cur_in, lng_bc[:])
                            cur_in = fv[:, n, :]
                        if not zero_lnb:
                            nc.vector.tensor_add(fv[:, n, :], cur_in, lnb_bc[:])
                        elif unit_lng:
                            nc.vector.tensor_copy(fv[:, n, :], cur_in)
                    fins.append(fv)
                o1 = sb.tile([P, SG, D], f32)
                nc.gpsimd.tensor_mul(o1[:], bccB[:], fins[0][:])
                o2 = sb.tile([P, SG, D], f32)
                nc.gpsimd.tensor_mul(o2[:], bmlB[:], fins[1][:])
                nc.gpsimd.tensor_add(ofinB[:], o1[:], o2[:])

            nc.sync.dma_start(
                out_d[rows, :].rearrange("(n p) d -> p n d", p=P), ofinB[:])

        if rep_cm is not None:
            rep_cm.__exit__(None, None, None)

    nc.compile()
    return nc


def _get_nc(flags, bs=BS):
    key = (flags, bs)
    if key not in _cache:
        _cache[key] = _build(flags, bs)
    return _cache[key]


def kernel(x_cc, x_mlo, view_mask, in_proj_w, in_proj_b, out_w, out_b,
           ln_g, ln_b, gate_w, gate_b):
    from concourse import bass_utils

    x_cc = np.asarray(x_cc, np.float32)
    x_mlo = np.asarray(x_mlo, np.float32)
    view_mask = np.asarray(view_mask, np.float32)
    in_proj_w = np.asarray(in_proj_w, np.float32)
    in_proj_b = np.asarray(in_proj_b, np.float32)
    out_w = np.asarray(out_w, np.float32)
    out_b = np.asarray(out_b, np.float32)
    ln_g = np.asarray(ln_g, np.float32)
    ln_b = np.asarray(ln_b, np.float32)
    gate_w = np.asarray(gate_w, np.float32)
    gate_b = np.asarray(gate_b, np.float32)

    # host-side weight prep (O(D^2), no per-row work)
    lng2 = np.concatenate([ln_g, ln_g])
    lnb2 = np.concatenate([ln_b, ln_b])
    gate_w_f = gate_w * lng2[None, :]
    gate_b_f = gate_b + gate_w @ lnb2
    Wq, Wk, Wv = in_proj_w[:D], in_proj_w[D:2 * D], in_proj_w[2 * D:]
    wqT8 = np.ascontiguousarray(Wq.T * WS).astype(F8)
    wkT8 = np.ascontiguousarray(Wk.T * WS).astype(F8)
    wvT = np.ascontiguousarray(Wv.T).astype(BF16)
    woT = np.ascontiguousarray(out_w.T).astype(BF16)
    gwT8 = np.ascontiguousarray(gate_w_f.T * WS).astype(F8)

    x_cc_bf = x_cc.astype(BF16)
    x_mlo_bf = x_mlo.astype(BF16)

    flags = (
        not in_proj_b.any(),
        not out_b.any(),
        not gate_b_f.any(),
        bool((ln_g == 1.0).all()),
        not ln_b.any(),
        bool(np.isin(view_mask, (0.0, 1.0)).all()),
    )
    nc = _get_nc(flags)

    in_maps = []
    for c in range(NCORES):
        sl = slice(c * BS, (c + 1) * BS)
        m = {
            "x_cc": x_cc_bf[sl], "x_mlo": x_mlo_bf[sl], "vm": view_mask[sl],
            "wqT8": wqT8, "wkT8": wkT8, "wvT": wvT, "woT": woT, "gwT8": gwT8,
        }
        zero_ipb, zero_ob, zero_gb2, unit_lng, zero_lnb, _ = flags
        if not zero_ipb:
            m["ipbq"] = in_proj_b[:D] * WS
            m["ipbv"] = in_proj_b[2 * D:]
        if not zero_ob:
            m["ob"] = out_b
        if not zero_gb2:
            m["gb2"] = gate_b_f * WS
        if not unit_lng:
            m["lng"] = ln_g
        if not zero_lnb:
            m["lnb"] = ln_b
        in_maps.append(m)

    global _last_run
    _last_run = (nc, in_maps)
    res = bass_utils.run_bass_kernel_spmd(nc, in_maps, core_ids=list(range(NCORES)))
    return np.concatenate([r["out"] for r in res.results],
                          axis=0).astype(np.float32)

